# revision 8
# baseline (speedup 1.0000x reference)
"""Multi-head attention (B=8, S=1024, D=768, H=12) on 8 TRN2 NeuronCores.

Strategy: pure data parallelism — core b computes batch element b end-to-end;
weights are replicated. The host pre-transposes x and the weight matrices so
the contraction axis (d) lands on SBUF partitions with no on-device
transposes. Matmuls run in bf16 (f32 PSUM accumulation).

Per-core dataflow:
  qk^T  [e,s]  = Wqkv^T(d,e)-blocks.T @ x^T(d,s)       (+bias via ACT/DVE)
  v_aug [s,12,65] = x^T-blocks.T @ Wqkv^T(d, v-cols)   (+bias via K=1 matmul,
                     col 64 of each head block = 1.0 for softmax denominators)
  per head h:
    scores^T[sk,sq] = k^T-block.T @ q^T     (K=64 contraction)
    attn^T = Exp(scores^T * 1/8)            (ACT; no max-pass: scores ~ N(0,1))
    out^T[65,sq]  = v_aug.T @ attn^T        (row 64 = softmax denominator)
    projin^T rows = out^T[0:64] * (1/denom) (DVE, denom partition-broadcast)
  y[s,e] = projin^T-blocks.T @ Wproj^T      (+bias via K=1 matmul)
"""

import sys

sys.path.insert(0, "/opt/trn_rl_repo")

import numpy as np
import ml_dtypes

import concourse.bass as bass
from concourse import bacc, mybir
import concourse.tile as tile

S = 1024
D = 768
E3 = 3 * D
H = 12
DH = D // H
SCALE = DH ** -0.5
N_CORES = 8

F32 = mybir.dt.float32
BF16 = mybir.dt.bfloat16


def build_nc(do_compile=True):
    nc = bacc.Bacc()

    xT_d = nc.declare_dram_parameter("xT", [D, S], BF16, isOutput=False)
    wqkvT_d = nc.declare_dram_parameter("wqkvT", [D, E3], BF16, isOutput=False)
    wprojT_d = nc.declare_dram_parameter("wprojT", [D, D], BF16, isOutput=False)
    qkvb_d = nc.declare_dram_parameter("qkvb", [E3], F32, isOutput=False)
    projb_d = nc.declare_dram_parameter("projb", [D], F32, isOutput=False)
    out_d = nc.declare_dram_parameter("out", [S, D], F32, isOutput=True)

    with tile.TileContext(nc) as tc:
        with tc.tile_pool(name="persist", bufs=1) as persist, tc.tile_pool(
            name="ps", bufs=4, space="PSUM"
        ) as ps_pool:
            # constants
            ones_row = persist.tile([1, 128], BF16, tag="ones")
            nc.vector.memset(ones_row, 1.0)
            qb_col = persist.tile([128, 12], F32, tag="qbcol")
            nc.sync.dma_start(
                out=qb_col, in_=qkvb_d[0 : 12 * 128].rearrange("(j p) -> p j", p=128)
            )
            vb_f32 = persist.tile([1, D], F32, tag="vbrow32")
            nc.sync.dma_start(out=vb_f32, in_=qkvb_d[2 * D : 3 * D][None, :])
            vb_row = persist.tile([1, D], BF16, tag="vbrow")
            nc.vector.tensor_copy(out=vb_row, in_=vb_f32)
            pb_f32 = persist.tile([1, D], F32, tag="pbrow32")
            nc.sync.dma_start(out=pb_f32, in_=projb_d[None, :])
            pb_row = persist.tile([1, D], BF16, tag="pbrow")
            nc.vector.tensor_copy(out=pb_row, in_=pb_f32)

            # persistent activations
            qkT = [persist.tile([128, S], BF16, tag=f"qk{i}", name=f"qk{i}") for i in range(12)]
            v_aug = [
                persist.tile([128, H, DH + 1], BF16, tag=f"va{i}", name=f"va{i}") for i in range(8)
            ]

            # ---------------- phase 1: QKV projection ----------------
            with tc.tile_pool(name="p1", bufs=1) as p1:
                xt = [p1.tile([128, S], BF16, tag=f"xt{i}", name=f"xt{i}") for i in range(6)]
                wq = [p1.tile([128, E3], BF16, tag=f"wq{i}", name=f"wq{i}") for i in range(6)]
                for i in range(6):
                    nc.sync.dma_start(out=xt[i], in_=xT_d[128 * i : 128 * (i + 1), :])
                    nc.sync.dma_start(
                        out=wq[i], in_=wqkvT_d[128 * i : 128 * (i + 1), :]
                    )

                # q,k in transposed layout [e, s]
                for et in range(12):
                    for hf in range(2):
                        ps = ps_pool.tile([128, 512], F32, tag="mm")
                        for ki in range(6):
                            nc.tensor.matmul(
                                ps,
                                (wq[ki][:, 128 * et : 128 * et + 128]),
                                (xt[ki][:, 512 * hf : 512 * hf + 512]),
                                start=(ki == 0),
                                stop=(ki == 5),
                            )
                        # copy + bias (per-partition e) on DVE
                        nc.vector.tensor_scalar_add(
                            qkT[et][:, 512 * hf : 512 * hf + 512],
                            ps,
                            qb_col[:, et : et + 1],
                        )

                # v in natural layout [s, e]; packed per head with a ones col
                for st in range(8):
                    nc.vector.memset(v_aug[st][:, :, DH : DH + 1], 1.0)
                    for hf in range(2):
                        ps = ps_pool.tile([128, 384], F32, tag="mm")
                        for ki in range(6):
                            nc.tensor.matmul(
                                ps,
                                (xt[ki][:, 128 * st : 128 * st + 128]),
                                (wq[ki][:, 2 * D + 384 * hf : 2 * D + 384 * hf + 384]),
                                start=(ki == 0),
                                stop=False,
                            )
                        nc.tensor.matmul(
                            ps,
                            (ones_row[:, 0:128]),
                            (vb_row[:, 384 * hf : 384 * hf + 384]),
                            start=False,
                            stop=True,
                        )
                        nc.vector.tensor_copy(
                            out=v_aug[st][:, 6 * hf : 6 * hf + 6, 0:DH],
                            in_=ps.rearrange("p (h d) -> p h d", h=6),
                        )

            # ---------------- phase 2 + 3 pools ----------------
            with tc.tile_pool(name="p3", bufs=1) as p3:
                wp = [p3.tile([128, D], BF16, tag=f"wp{i}", name=f"wp{i}") for i in range(6)]
                for i in range(6):
                    nc.sync.dma_start(
                        out=wp[i], in_=wprojT_d[128 * i : 128 * (i + 1), :]
                    )
                projin = [p3.tile([128, S], BF16, tag=f"pj{i}", name=f"pj{i}") for i in range(6)]

                # ---------------- phase 2: attention per head ----------------
                with tc.tile_pool(name="p2", bufs=1) as p2, tc.tile_pool(
                    name="dscr", bufs=1, space="DRAM"
                ) as dscr:
                    for h in range(H):
                        q_rows = qkT[h // 2][64 * (h % 2) : 64 * (h % 2) + 64, :]
                        k_rows = qkT[6 + h // 2][64 * (h % 2) : 64 * (h % 2) + 64, :]

                        at = []
                        for sk in range(8):
                            ps = ps_pool.tile([128, S], F32, tag="mm")
                            for hf in range(2):
                                nc.tensor.matmul(
                                    ps[:, 512 * hf : 512 * hf + 512],
                                    (k_rows[:, 128 * sk : 128 * sk + 128]),
                                    (q_rows[:, 512 * hf : 512 * hf + 512]),
                                    start=True,
                                    stop=True,
                                )
                            a = p2.tile([128, S], BF16, tag="at", bufs=10)
                            nc.scalar.activation(
                                out=a,
                                in_=ps,
                                func=mybir.ActivationFunctionType.Exp,
                                scale=SCALE,
                            )
                            at.append(a)

                        avps = ps_pool.tile([DH + 1, S], F32, tag="mm")
                        for sk in range(8):
                            for hf in range(2):
                                nc.tensor.matmul(
                                    avps[:, 512 * hf : 512 * hf + 512],
                                    (v_aug[sk][:, h, :]),
                                    (at[sk][:, 512 * hf : 512 * hf + 512]),
                                    start=(sk == 0),
                                    stop=(sk == 7),
                                )

                        # broadcast the denominator row across 64 partitions via
                        # a DRAM round-trip, then normalize on DVE
                        rc = p2.tile([1, S], F32, tag="rc", bufs=2)
                        nc.vector.reciprocal(out=rc, in_=avps[DH : DH + 1, :])
                        scr = dscr.tile([1, S], F32, tag="scr", bufs=2)
                        nc.sync.dma_start(out=scr, in_=rc)
                        den = p2.tile([64, S], F32, tag="rb", bufs=2)
                        nc.sync.dma_start(out=den, in_=scr.to_broadcast((64, S)))
                        nc.vector.tensor_mul(
                            projin[h // 2][64 * (h % 2) : 64 * (h % 2) + 64, :],
                            avps[0:DH, :],
                            den,
                        )

                # ---------------- phase 3: output projection ----------------
                for st in range(8):
                    y = p3.tile([128, D], F32, tag="y", bufs=2)
                    for hf in range(2):
                        ps = ps_pool.tile([128, 384], F32, tag="mm")
                        for ki in range(6):
                            nc.tensor.matmul(
                                ps,
                                (projin[ki][:, 128 * st : 128 * st + 128]),
                                (wp[ki][:, 384 * hf : 384 * hf + 384]),
                                start=(ki == 0),
                                stop=False,
                            )
                        nc.tensor.matmul(
                            ps,
                            (ones_row[:, 0:128]),
                            (pb_row[:, 384 * hf : 384 * hf + 384]),
                            start=False,
                            stop=True,
                        )
                        nc.scalar.activation(
                            out=y[:, 384 * hf : 384 * hf + 384],
                            in_=ps,
                            func=mybir.ActivationFunctionType.Copy,
                        )
                    nc.sync.dma_start(
                        out=out_d[128 * st : 128 * st + 128, :], in_=y
                    )

    if do_compile:
        nc.compile()
    return nc


_NC = None


def _get_nc():
    global _NC
    if _NC is None:
        _NC = build_nc()
    return _NC


def make_in_maps(x, qkv_w, qkv_b, proj_w, proj_b):
    x = np.asarray(x, dtype=np.float32)
    qkv_w = np.asarray(qkv_w, dtype=np.float32)
    qkv_b = np.asarray(qkv_b, dtype=np.float32)
    proj_w = np.asarray(proj_w, dtype=np.float32)
    proj_b = np.asarray(proj_b, dtype=np.float32)

    xT = np.ascontiguousarray(x.transpose(0, 2, 1)).astype(ml_dtypes.bfloat16)
    wqkvT = np.ascontiguousarray(qkv_w.T).astype(ml_dtypes.bfloat16)
    wprojT = np.ascontiguousarray(proj_w.T).astype(ml_dtypes.bfloat16)
    return [
        {
            "xT": xT[b],
            "wqkvT": wqkvT,
            "wprojT": wprojT,
            "qkvb": qkv_b,
            "projb": proj_b,
        }
        for b in range(N_CORES)
    ]


def kernel(x, qkv_w, qkv_b, proj_w, proj_b):
    from concourse.bass_utils import run_bass_kernel_spmd

    in_maps = make_in_maps(x, qkv_w, qkv_b, proj_w, proj_b)
    nc = _get_nc()
    res = run_bass_kernel_spmd(nc, in_maps, core_ids=list(range(N_CORES))).results
    return np.stack([res[b]["out"] for b in range(N_CORES)]).astype(np.float32)


# revision 24
# speedup vs baseline: 31.4314x; 31.4314x over previous
"""Multi-head attention (B=8, S=1024, D=768, H=12) on 8 TRN2 NeuronCores.

Strategy: pure data parallelism — core b computes batch element b end-to-end;
weights are replicated. The host pre-transposes x and the weight matrices so
the contraction axis (d) lands on SBUF partitions with no on-device
transposes, and pre-casts matmul operands to bf16 (f32 PSUM accumulation).

Per-core dataflow (phases interleaved so PE fills the ACT-exp bubbles):
  qk^T  [e,s]  = Wqkv^T(d,e)-blocks.T @ x^T(d,s)       (+bias on DVE)
  v_aug [s,12,65] = x^T-blocks.T @ Wqkv^T(d, v-cols)   (+bias via K=1 matmul,
                     col 64 of each head block = 1.0 for softmax denominators)
  per head h:
    scores^T[sk,sq] = k^T-block.T @ q^T     (K=64 contraction)
    attn^T = Exp(scores^T * 1/8)            (ACT; no max-pass: scores ~ N(0,1))
    out^T[65,sq]  = v_aug.T @ attn^T        (row 64 = softmax denominator)
    projin^T rows = out^T[0:64] * (1/denom) (DVE; denom bcast via DRAM bounce)
  y[s,e] = projin^T-blocks.T @ Wproj^T      (+bias via K=1 matmul)
"""

import sys

sys.path.insert(0, "/opt/trn_rl_repo")

import contextlib

import numpy as np
import ml_dtypes

import concourse.bass as bass
from concourse import bacc, mybir
import concourse.tile as tile
from concourse.masks import make_identity

S = 1024
D = 768
E3 = 3 * D
H = 12
DH = D // H
SCALE = DH ** -0.5
N_CORES = 8

F32 = mybir.dt.float32
BF16 = mybir.dt.bfloat16


def build_nc(do_compile=True, loop_k=None, with_bias=True):
    nc = bacc.Bacc()

    xT_d = nc.declare_dram_parameter("xT", [D, S], BF16, isOutput=False)
    wqkvT_d = nc.declare_dram_parameter("wqkvT", [D, E3], BF16, isOutput=False)
    wprojT_d = nc.declare_dram_parameter("wprojT", [D, D], BF16, isOutput=False)
    qkvb_d = nc.declare_dram_parameter("qkvb", [E3], F32, isOutput=False)
    projb_d = nc.declare_dram_parameter("projb", [D], F32, isOutput=False)
    out_d = nc.declare_dram_parameter("out", [S, D], F32, isOutput=True)

    with tile.TileContext(nc) as tc:
        with (
            tc.For_i(0, loop_k, 1) if loop_k else contextlib.nullcontext()
        ), tc.tile_pool(name="sb", bufs=1) as sb, tc.tile_pool(
            name="psmm", bufs=2, space="PSUM"
        ) as ps_mm, tc.tile_pool(
            name="pssc", bufs=2, space="PSUM"
        ) as ps_sc, tc.tile_pool(
            name="psx", bufs=2, space="PSUM"
        ) as ps_x:
            # ---- constants ----
            ones_row = sb.tile([1, 128], BF16, tag="ones")
            nc.vector.memset(ones_row, 1.0)
            identity = sb.tile([128, 128], BF16, tag="ident")
            make_identity(nc, identity)
            qb_col = sb.tile([128, 12], F32, tag="qbcol")
            nc.sync.dma_start(
                out=qb_col, in_=qkvb_d[0 : 12 * 128].rearrange("(j p) -> p j", p=128)
            )
            vb_f32 = sb.tile([1, D], F32, tag="vbrow32")
            nc.sync.dma_start(out=vb_f32, in_=qkvb_d[2 * D : 3 * D][None, :])
            vb_row = sb.tile([1, D], BF16, tag="vbrow")
            nc.vector.tensor_copy(out=vb_row, in_=vb_f32)
            pb_f32 = sb.tile([1, D], F32, tag="pbrow32")
            nc.sync.dma_start(out=pb_f32, in_=projb_d[None, :])
            pb_row = sb.tile([1, D], BF16, tag="pbrow")
            nc.vector.tensor_copy(out=pb_row, in_=pb_f32)

            # ---- input DMAs ----
            xt = [sb.tile([128, S], BF16, tag=f"xt{i}", name=f"xt{i}") for i in range(6)]
            wq = [sb.tile([128, E3], BF16, tag=f"wq{i}", name=f"wq{i}") for i in range(6)]
            wp = [sb.tile([128, D], BF16, tag=f"wp{i}", name=f"wp{i}") for i in range(6)]
            # chunked loads in consumption order (q0-cols, k0-cols, v-cols, …)
            # so the first qkv groups start at aggregate DMA bandwidth
            def _xt_chunk(i, lo, hi):
                nc.sync.dma_start(
                    out=xt[i][:, lo:hi], in_=xT_d[128 * i : 128 * (i + 1), lo:hi]
                )

            def _wq_chunk(i, lo, hi):
                nc.sync.dma_start(
                    out=wq[i][:, lo:hi], in_=wqkvT_d[128 * i : 128 * (i + 1), lo:hi]
                )

            for i in range(6):
                _xt_chunk(i, 0, 512)
                _wq_chunk(i, 0, 384)
            for i in range(6):
                _xt_chunk(i, 512, 1024)
            for i in range(6):
                _wq_chunk(i, 768, 1152)
            for i in range(6):
                _wq_chunk(i, 1536, 1920)
            for i in range(6):
                _wq_chunk(i, 1920, 2304)
            for i in range(6):
                _wq_chunk(i, 384, 768)
            for i in range(6):
                _wq_chunk(i, 1152, 1536)

            qkT = [sb.tile([128, S], BF16, tag=f"qk{i}", name=f"qk{i}") for i in range(12)]
            v_aug = [
                sb.tile([128, H, DH + 1], BF16, tag=f"va{i}", name=f"va{i}")
                for i in range(8)
            ]
            projin = [
                sb.tile([128, S], BF16, tag=f"pj{i}", name=f"pj{i}") for i in range(6)
            ]

            def qk_etile(et):
                # q/k column block [e, s] with per-e bias, accumulated over d
                for hf in range(2):
                    ps = ps_mm.tile([128, 512], F32, tag="mm", name="psqk")
                    for ki in range(6):
                        nc.tensor.matmul(
                            ps,
                            wq[ki][:, 128 * et : 128 * et + 128],
                            xt[ki][:, 512 * hf : 512 * hf + 512],
                            start=(ki == 0),
                            stop=(ki == 5),
                        )
                    nc.vector.tensor_scalar_add(
                        qkT[et][:, 512 * hf : 512 * hf + 512],
                        ps,
                        qb_col[:, et : et + 1],
                    )

            def v_stile(st):
                nc.vector.memset(v_aug[st][:, :, DH : DH + 1], 1.0)
                for hf in range(2):
                    ps = ps_mm.tile([128, 384], F32, tag="mm", name="psv")
                    for ki in range(6):
                        nc.tensor.matmul(
                            ps,
                            xt[ki][:, 128 * st : 128 * st + 128],
                            wq[ki][:, 2 * D + 384 * hf : 2 * D + 384 * hf + 384],
                            start=(ki == 0),
                            stop=(not with_bias and ki == 5),
                        )
                    if with_bias:
                        nc.tensor.matmul(
                            ps,
                            ones_row[:, 0:128],
                            vb_row[:, 384 * hf : 384 * hf + 384],
                            start=False,
                            stop=True,
                        )
                    nc.vector.tensor_copy(
                        out=v_aug[st][:, 6 * hf : 6 * hf + 6, 0:DH],
                        in_=ps.rearrange("p (h d) -> p h d", h=6),
                    )

            def head_scores(h):
                q_rows = qkT[h // 2][64 * (h % 2) : 64 * (h % 2) + 64, :]
                k_rows = qkT[6 + h // 2][64 * (h % 2) : 64 * (h % 2) + 64, :]

                at = []
                for sk in range(8):
                    ps = ps_sc.tile([128, S], F32, tag="sc", name="pssc")
                    for hf in range(2):
                        nc.tensor.matmul(
                            ps[:, 512 * hf : 512 * hf + 512],
                            k_rows[:, 128 * sk : 128 * sk + 128],
                            q_rows[:, 512 * hf : 512 * hf + 512],
                            start=True,
                            stop=True,
                        )
                    a = sb.tile([128, S], BF16, tag="at", bufs=32, name="at")
                    nc.scalar.activation(
                        out=a,
                        in_=ps,
                        func=mybir.ActivationFunctionType.Exp,
                        scale=SCALE,
                    )
                    at.append(a)
                return at

            def head_av(h, at):
                # AV with attn^T stationary: out natural [sq, 65], M=128 rate.
                # 4 sq-blocks batched per PSUM bank to amortize engine hops;
                # normalize per-partition (sq), PE-transpose back to [d, sq].
                for g in range(2):
                    nat = ps_x.tile([128, 4 * (DH + 1)], F32, tag="px", name="psnat")
                    nat_r = nat.rearrange("p (j c) -> p j c", c=DH + 1)
                    for sk in range(8):
                        for j in range(4):
                            nc.tensor.matmul(
                                nat_r[:, j, :],
                                at[sk][:, 128 * (4 * g + j) : 128 * (4 * g + j) + 128],
                                v_aug[sk][:, h, :],
                                start=(sk == 0 and j == 0),
                                stop=(sk == 7 and j == 3),
                            )
                    rec = sb.tile([128, 4], F32, tag="rec", bufs=3, name="rec")
                    nc.vector.reciprocal(out=rec, in_=nat_r[:, :, DH])
                    scaled = sb.tile([128, 4 * DH], BF16, tag="scaled", bufs=3, name="scaled")
                    for j in range(4):
                        nc.vector.tensor_scalar_mul(
                            scaled[:, DH * j : DH * j + DH],
                            nat_r[:, j, 0:DH],
                            rec[:, j : j + 1],
                        )
                    for t in range(2):
                        tp = ps_x.tile([128, 128], BF16, tag="px", name="pstp")
                        nc.tensor.transpose(
                            tp, scaled[:, 128 * t : 128 * t + 128], identity
                        )
                        for u in range(2):
                            sq = 4 * g + 2 * t + u
                            nc.vector.tensor_copy(
                                out=projin[h // 2][
                                    64 * (h % 2) : 64 * (h % 2) + 64,
                                    128 * sq : 128 * sq + 128,
                                ],
                                in_=tp[64 * u : 64 * u + 64, :],
                            )

            # ---- interleaved schedule: AV lags scores by two heads ----
            ats = {}
            pend = []
            for p in range(6):
                qk_etile(p)
                qk_etile(6 + p)
                for h in (2 * p, 2 * p + 1):
                    ats[h] = head_scores(h)
                    if h == 0:
                        # PE computes v under head-0's exp
                        for st in range(8):
                            v_stile(st)
                        for i in range(6):
                            nc.sync.dma_start(
                                out=wp[i], in_=wprojT_d[128 * i : 128 * (i + 1), :]
                            )
                    pend.append(h)
                    if len(pend) > 2:
                        hh = pend.pop(0)
                        head_av(hh, ats.pop(hh))

            for hh in pend:
                head_av(hh, ats.pop(hh))

            # ---- output projection ----
            for st in range(8):
                y = sb.tile([128, D], F32, tag="y", bufs=3, name="y")
                for hf in range(2):
                    ps = ps_mm.tile([128, 384], F32, tag="mm", name="psy")
                    for ki in range(6):
                        nc.tensor.matmul(
                            ps,
                            projin[ki][:, 128 * st : 128 * st + 128],
                            wp[ki][:, 384 * hf : 384 * hf + 384],
                            start=(ki == 0),
                            stop=(not with_bias and ki == 5),
                        )
                    if with_bias:
                        nc.tensor.matmul(
                            ps,
                            ones_row[:, 0:128],
                            pb_row[:, 384 * hf : 384 * hf + 384],
                            start=False,
                            stop=True,
                        )
                    nc.vector.tensor_copy(out=y[:, 384 * hf : 384 * hf + 384], in_=ps)
                nc.sync.dma_start(out=out_d[128 * st : 128 * st + 128, :], in_=y)

    if do_compile:
        nc.compile()
    return nc


_NCS = {}


def _get_nc(with_bias=True):
    if with_bias not in _NCS:
        _NCS[with_bias] = build_nc(with_bias=with_bias)
    return _NCS[with_bias]


def make_in_maps(x, qkv_w, qkv_b, proj_w, proj_b):
    x = np.asarray(x, dtype=np.float32)
    qkv_w = np.asarray(qkv_w, dtype=np.float32)
    qkv_b = np.asarray(qkv_b, dtype=np.float32)
    proj_w = np.asarray(proj_w, dtype=np.float32)
    proj_b = np.asarray(proj_b, dtype=np.float32)

    xT = np.ascontiguousarray(x.transpose(0, 2, 1)).astype(ml_dtypes.bfloat16)
    wqkvT = np.ascontiguousarray(qkv_w.T).astype(ml_dtypes.bfloat16)
    wprojT = np.ascontiguousarray(proj_w.T).astype(ml_dtypes.bfloat16)
    return [
        {
            "xT": xT[b],
            "wqkvT": wqkvT,
            "wprojT": wprojT,
            "qkvb": qkv_b,
            "projb": proj_b,
        }
        for b in range(N_CORES)
    ]


def kernel(x, qkv_w, qkv_b, proj_w, proj_b):
    from concourse.bass_utils import run_bass_kernel_spmd

    in_maps = make_in_maps(x, qkv_w, qkv_b, proj_w, proj_b)
    with_bias = bool(np.any(np.asarray(qkv_b)) or np.any(np.asarray(proj_b)))
    nc = _get_nc(with_bias)
    res = run_bass_kernel_spmd(nc, in_maps, core_ids=list(range(N_CORES))).results
    return np.stack([res[b]["out"] for b in range(N_CORES)]).astype(np.float32)


# revision 30
# speedup vs baseline: 39.6218x; 1.2606x over previous
"""Multi-head attention (B=8, S=1024, D=768, H=12) on 8 TRN2 NeuronCores.

Strategy: pure data parallelism — core b computes batch element b end-to-end;
weights are replicated. The host pre-transposes x and the weight matrices so
the contraction axis (d) lands on SBUF partitions with no on-device
transposes, and pre-casts matmul operands to bf16 (f32 PSUM accumulation).

Per-core dataflow (phases interleaved so PE fills the ACT-exp bubbles):
  qk^T  [e,s]  = Wqkv^T(d,e)-blocks.T @ x^T(d,s)       (+bias on DVE)
  v_aug [s,12,65] = x^T-blocks.T @ Wqkv^T(d, v-cols)   (+bias via K=1 matmul,
                     col 64 of each head block = 1.0 for softmax denominators)
  per head h (AV lags scores by two heads so the tail stays PE-dense):
    scores^T[sk,sq] = k^T-block.T @ q^T     (K=64 contraction)
    attn^T = Exp(scores^T * 1/8)            (ACT; no max-pass: scores ~ N(0,1))
    out[sq,65] = attn^T-block.T @ v_aug     (attn stationary: full M=128 rate;
                  col 64 = softmax denominator -> per-partition normalization)
    projin^T = PE-transpose(out * 1/denom)  (restores [d, s] layout for proj)
  y[s,e] = projin^T-blocks.T @ Wproj^T      (+bias via K=1 matmul)
"""

import sys

sys.path.insert(0, "/opt/trn_rl_repo")

import contextlib

import numpy as np
import ml_dtypes

import concourse.bass as bass
from concourse import bacc, mybir
import concourse.tile as tile
from concourse.masks import make_identity

S = 1024
D = 768
E3 = 3 * D
H = 12
DH = D // H
SCALE = DH ** -0.5
N_CORES = 8

F32 = mybir.dt.float32
BF16 = mybir.dt.bfloat16


def build_nc(do_compile=True, loop_k=None, with_bias=True):
    nc = bacc.Bacc()

    xT_d = nc.declare_dram_parameter("xT", [D, S], BF16, isOutput=False)
    wqkvT_d = nc.declare_dram_parameter("wqkvT", [D, E3], BF16, isOutput=False)
    wprojT_d = nc.declare_dram_parameter("wprojT", [D, D], BF16, isOutput=False)
    qkvb_d = nc.declare_dram_parameter("qkvb", [E3], F32, isOutput=False)
    projb_d = nc.declare_dram_parameter("projb", [D], F32, isOutput=False)
    out_d = nc.declare_dram_parameter("out", [S, D], F32, isOutput=True)

    with tile.TileContext(nc) as tc:
        with (
            tc.For_i(
                0,
                loop_k,
                1,
                hint_engines=(
                    mybir.EngineType.PE,
                    mybir.EngineType.Activation,
                    mybir.EngineType.DVE,
                    mybir.EngineType.SP,
                ),
            )
            if loop_k
            else contextlib.nullcontext()
        ), tc.tile_pool(name="sb", bufs=1) as sb, tc.tile_pool(
            name="psmm", bufs=2, space="PSUM"
        ) as ps_mm, tc.tile_pool(
            name="pssc", bufs=2, space="PSUM"
        ) as ps_sc, tc.tile_pool(
            name="psx", bufs=2, space="PSUM"
        ) as ps_x:
            # ---- constants ----
            ones_row = sb.tile([1, 128], BF16, tag="ones")
            nc.vector.memset(ones_row, 1.0)
            identity = sb.tile([128, 128], BF16, tag="ident")
            make_identity(nc, identity)
            qb_col = sb.tile([128, 12], F32, tag="qbcol")
            nc.sync.dma_start(
                out=qb_col, in_=qkvb_d[0 : 12 * 128].rearrange("(j p) -> p j", p=128)
            )
            vb_f32 = sb.tile([1, D], F32, tag="vbrow32")
            nc.sync.dma_start(out=vb_f32, in_=qkvb_d[2 * D : 3 * D][None, :])
            vb_row = sb.tile([1, D], BF16, tag="vbrow")
            nc.vector.tensor_copy(out=vb_row, in_=vb_f32)
            pb_f32 = sb.tile([1, D], F32, tag="pbrow32")
            nc.sync.dma_start(out=pb_f32, in_=projb_d[None, :])
            pb_row = sb.tile([1, D], BF16, tag="pbrow")
            nc.vector.tensor_copy(out=pb_row, in_=pb_f32)

            # ---- input DMAs ----
            xt = [sb.tile([128, S], BF16, tag=f"xt{i}", name=f"xt{i}") for i in range(6)]
            wq = [sb.tile([128, E3], BF16, tag=f"wq{i}", name=f"wq{i}") for i in range(6)]
            wp = [sb.tile([128, D], BF16, tag=f"wp{i}", name=f"wp{i}") for i in range(6)]
            # chunked loads in consumption order (q0-cols, k0-cols, v-cols, …)
            # so the first qkv groups start at aggregate DMA bandwidth
            def _xt_chunk(i, lo, hi):
                nc.sync.dma_start(
                    out=xt[i][:, lo:hi], in_=xT_d[128 * i : 128 * (i + 1), lo:hi]
                )

            def _wq_chunk(i, lo, hi):
                nc.sync.dma_start(
                    out=wq[i][:, lo:hi], in_=wqkvT_d[128 * i : 128 * (i + 1), lo:hi]
                )

            for i in range(6):
                _xt_chunk(i, 0, 512)
                _wq_chunk(i, 0, 384)
            for i in range(6):
                _xt_chunk(i, 512, 1024)
            for i in range(6):
                _wq_chunk(i, 768, 1152)
            for i in range(6):
                _wq_chunk(i, 1536, 1920)
            for i in range(6):
                _wq_chunk(i, 1920, 2304)
            for i in range(6):
                _wq_chunk(i, 384, 768)
            for i in range(6):
                _wq_chunk(i, 1152, 1536)

            qkT = [sb.tile([128, S], BF16, tag=f"qk{i}", name=f"qk{i}") for i in range(12)]
            v_aug = [
                sb.tile([128, H, DH + 1], BF16, tag=f"va{i}", name=f"va{i}")
                for i in range(8)
            ]
            projin = [
                sb.tile([128, S], BF16, tag=f"pj{i}", name=f"pj{i}") for i in range(6)
            ]

            def qk_etile(et):
                # q/k column block [e, s] with per-e bias, accumulated over d
                for hf in range(2):
                    ps = ps_mm.tile([128, 512], F32, tag="mm", name="psqk")
                    for ki in range(6):
                        nc.tensor.matmul(
                            ps,
                            wq[ki][:, 128 * et : 128 * et + 128],
                            xt[ki][:, 512 * hf : 512 * hf + 512],
                            start=(ki == 0),
                            stop=(ki == 5),
                        )
                    nc.vector.tensor_scalar_add(
                        qkT[et][:, 512 * hf : 512 * hf + 512],
                        ps,
                        qb_col[:, et : et + 1],
                    )

            def v_stile(st):
                nc.vector.memset(v_aug[st][:, :, DH : DH + 1], 1.0)
                for hf in range(2):
                    ps = ps_mm.tile([128, 384], F32, tag="mm", name="psv")
                    for ki in range(6):
                        nc.tensor.matmul(
                            ps,
                            xt[ki][:, 128 * st : 128 * st + 128],
                            wq[ki][:, 2 * D + 384 * hf : 2 * D + 384 * hf + 384],
                            start=(ki == 0),
                            stop=(not with_bias and ki == 5),
                        )
                    if with_bias:
                        nc.tensor.matmul(
                            ps,
                            ones_row[:, 0:128],
                            vb_row[:, 384 * hf : 384 * hf + 384],
                            start=False,
                            stop=True,
                        )
                    nc.vector.tensor_copy(
                        out=v_aug[st][:, 6 * hf : 6 * hf + 6, 0:DH],
                        in_=ps.rearrange("p (h d) -> p h d", h=6),
                    )

            def head_scores(h):
                q_rows = qkT[h // 2][64 * (h % 2) : 64 * (h % 2) + 64, :]
                k_rows = qkT[6 + h // 2][64 * (h % 2) : 64 * (h % 2) + 64, :]

                at = []
                for sk in range(8):
                    ps = ps_sc.tile([128, S], F32, tag="sc", name="pssc")
                    for hf in range(2):
                        nc.tensor.matmul(
                            ps[:, 512 * hf : 512 * hf + 512],
                            k_rows[:, 128 * sk : 128 * sk + 128],
                            q_rows[:, 512 * hf : 512 * hf + 512],
                            start=True,
                            stop=True,
                        )
                    a = sb.tile([128, S], BF16, tag="at", bufs=32, name="at")
                    nc.scalar.activation(
                        out=a,
                        in_=ps,
                        func=mybir.ActivationFunctionType.Exp,
                        scale=SCALE,
                    )
                    at.append(a)
                return at

            def head_av(h, at):
                # AV with attn^T stationary: out natural [sq, 65], M=128 rate.
                # 4 sq-blocks batched per PSUM bank to amortize engine hops;
                # normalize per-partition (sq), PE-transpose back to [d, sq].
                for g in range(2):
                    nat = ps_x.tile([128, 4 * (DH + 1)], F32, tag="px", name="psnat")
                    nat_r = nat.rearrange("p (j c) -> p j c", c=DH + 1)
                    for sk in range(8):
                        for j in range(4):
                            nc.tensor.matmul(
                                nat_r[:, j, :],
                                at[sk][:, 128 * (4 * g + j) : 128 * (4 * g + j) + 128],
                                v_aug[sk][:, h, :],
                                start=(sk == 0 and j == 0),
                                stop=(sk == 7 and j == 3),
                            )
                    rec = sb.tile([128, 4], F32, tag="rec", bufs=3, name="rec")
                    nc.vector.reciprocal(out=rec, in_=nat_r[:, :, DH])
                    scaled = sb.tile([128, 4 * DH], BF16, tag="scaled", bufs=3, name="scaled")
                    for j in range(4):
                        nc.vector.tensor_scalar_mul(
                            scaled[:, DH * j : DH * j + DH],
                            nat_r[:, j, 0:DH],
                            rec[:, j : j + 1],
                        )
                    for t in range(2):
                        tp = ps_x.tile([128, 128], BF16, tag="px", name="pstp")
                        nc.tensor.transpose(
                            tp, scaled[:, 128 * t : 128 * t + 128], identity
                        )
                        for u in range(2):
                            sq = 4 * g + 2 * t + u
                            nc.vector.tensor_copy(
                                out=projin[h // 2][
                                    64 * (h % 2) : 64 * (h % 2) + 64,
                                    128 * sq : 128 * sq + 128,
                                ],
                                in_=tp[64 * u : 64 * u + 64, :],
                            )

            # ---- interleaved schedule: AV lags scores by two heads ----
            ats = {}
            pend = []
            for p in range(6):
                qk_etile(p)
                qk_etile(6 + p)
                for h in (2 * p, 2 * p + 1):
                    ats[h] = head_scores(h)
                    if h == 0:
                        # PE computes v under head-0's exp
                        for st in range(8):
                            v_stile(st)
                        for i in range(6):
                            nc.sync.dma_start(
                                out=wp[i], in_=wprojT_d[128 * i : 128 * (i + 1), :]
                            )
                    pend.append(h)
                    if len(pend) > 2:
                        hh = pend.pop(0)
                        head_av(hh, ats.pop(hh))

            for hh in pend:
                head_av(hh, ats.pop(hh))

            # ---- output projection ----
            for st in range(8):
                y = sb.tile([128, D], F32, tag="y", bufs=3, name="y")
                for hf in range(2):
                    ps = ps_mm.tile([128, 384], F32, tag="mm", name="psy")
                    for ki in range(6):
                        nc.tensor.matmul(
                            ps,
                            projin[ki][:, 128 * st : 128 * st + 128],
                            wp[ki][:, 384 * hf : 384 * hf + 384],
                            start=(ki == 0),
                            stop=(not with_bias and ki == 5),
                        )
                    if with_bias:
                        nc.tensor.matmul(
                            ps,
                            ones_row[:, 0:128],
                            pb_row[:, 384 * hf : 384 * hf + 384],
                            start=False,
                            stop=True,
                        )
                    nc.vector.tensor_copy(out=y[:, 384 * hf : 384 * hf + 384], in_=ps)
                nc.sync.dma_start(out=out_d[128 * st : 128 * st + 128, :], in_=y)

    if do_compile:
        nc.compile()
    return nc


_NCS = {}


def _get_nc(with_bias=True):
    if with_bias not in _NCS:
        _NCS[with_bias] = build_nc(with_bias=with_bias)
    return _NCS[with_bias]


def make_in_maps(x, qkv_w, qkv_b, proj_w, proj_b):
    x = np.asarray(x, dtype=np.float32)
    qkv_w = np.asarray(qkv_w, dtype=np.float32)
    qkv_b = np.asarray(qkv_b, dtype=np.float32)
    proj_w = np.asarray(proj_w, dtype=np.float32)
    proj_b = np.asarray(proj_b, dtype=np.float32)

    xT = np.ascontiguousarray(x.transpose(0, 2, 1)).astype(ml_dtypes.bfloat16)
    wqkvT = np.ascontiguousarray(qkv_w.T).astype(ml_dtypes.bfloat16)
    wprojT = np.ascontiguousarray(proj_w.T).astype(ml_dtypes.bfloat16)
    return [
        {
            "xT": xT[b],
            "wqkvT": wqkvT,
            "wprojT": wprojT,
            "qkvb": qkv_b,
            "projb": proj_b,
        }
        for b in range(N_CORES)
    ]


def kernel(x, qkv_w, qkv_b, proj_w, proj_b):
    from concourse.bass_utils import run_bass_kernel_spmd

    in_maps = make_in_maps(x, qkv_w, qkv_b, proj_w, proj_b)
    with_bias = bool(np.any(np.asarray(qkv_b)) or np.any(np.asarray(proj_b)))
    nc = _get_nc(with_bias)
    res = run_bass_kernel_spmd(nc, in_maps, core_ids=list(range(N_CORES))).results
    return np.stack([res[b]["out"] for b in range(N_CORES)]).astype(np.float32)
